# revision 10
# baseline (speedup 1.0000x reference)
"""Depthwise 5x5 correlation (stride 1, pad 2) over X[4, 32, 512, 512] fp32,
with a single shared [5, 5] kernel, on 8 Trainium2 NeuronCores.

Strategy (pure data parallel): the 4*32 = 128 images are split 16 per core.
The input is zero-padded host-side to [516, 516] (pad 2 in H and W), so on
device the conv decomposes per kernel column j:
    O[h, w] = sum_j C_j[h, w],   C_j[h, w] = sum_k B_j[k, h] X'[h + k, w + j]
where B_j is a single banded-Toeplitz stationary matrix (B_j[k, m] =
kernel[k - m, j]); one TensorE matmul per (row-block, j), all five j's
accumulating into the same PSUM bank (start=True on j=0 zero-fills it), with
the W shift folded into the rhs read offset.

H is tiled into 4 uniform blocks of 124 output rows (each reading 128 padded
input rows) plus one 16-row edge block (reading 20 padded rows). The four
uniform blocks of an image share one SBUF output tile [124, 4, 512] written
back with a single ~1 MB DMA whose descriptors spread across all 16 SDMA
engines; the 16-row edges of all images are batched into one global in-DMA
and one global out-DMA. DMA issue alternates between the SP and ACT HWDGE
rings to parallelize queue-push overhead.

Matmuls run as float32r (single-pass relaxed fp32, 4x faster than strict fp32
on the PE, fp32 PSUM accumulate).
"""

import numpy as np

import concourse.bacc as bacc
import concourse.bass as bass
import concourse.mybir as mybir
import concourse.tile as tile
from concourse.bass_utils import run_bass_kernel_spmd

F32 = mybir.dt.float32
F32R = mybir.dt.float32r

N_CORES = 8
IMGS_PER_CORE = 16
H = W = 512
HP = H + 4
WP = W + 4
KS = 5

NB = 4           # uniform row blocks per image
MB = 124         # output rows per uniform block
ME = 16          # output rows in the edge block (rows 496..512)
KE = ME + KS - 1  # padded input rows the edge block reads

USE_F32R = True

_CACHE = {}


def build_bands(kern):
    """kern: [5, 5] fp32 -> [128, 5, 124] banded-Toeplitz stationary matrices,
    partition-major. B[k, j, m] = kern[k - m, j] for k - m in [0, 5).
    The edge block uses the [:20, :, :16] slice (same band structure)."""
    kern = np.asarray(kern, dtype=np.float32)
    B = np.zeros((MB + 4, KS, MB), dtype=np.float32)
    k_idx = np.arange(MB + 4)[:, None]
    m_idx = np.arange(MB)[None, :]
    tap = k_idx - m_idx
    valid = (tap >= 0) & (tap < KS)
    kk, mm = np.nonzero(valid)
    for j in range(KS):
        B[kk, j, mm] = kern[tap[kk, mm], j]
    return B


def build_nc():
    # float32r end-to-end on the matmul operand path (DRAM declaration, DMA,
    # SBUF tile, matmul input): walrus' BIR verifier requires the producer of
    # an FP32r matmul operand to emit FP32r. Same 4-byte fp32 bits on the wire.
    mm_dt = F32R if USE_F32R else F32
    nc = bacc.Bacc("TRN2", target_bir_lowering=False, debug=False)

    x = nc.dram_tensor("x", [IMGS_PER_CORE, HP, WP], mm_dt, kind="ExternalInput").ap()
    bm = nc.dram_tensor("bm", [MB + 4, KS, MB], mm_dt, kind="ExternalInput").ap()
    y = nc.dram_tensor("y", [IMGS_PER_CORE, H, W], F32, kind="ExternalOutput").ap()
    xh = x.tensor  # handle for raw-AP construction
    yh = y.tensor

    with tile.TileContext(nc) as tc:
        with (
            tc.tile_pool(name="bands", bufs=1) as bpool,
            tc.tile_pool(name="xin", bufs=12) as xpool,
            tc.tile_pool(name="edge", bufs=1) as epool,
            tc.tile_pool(name="out", bufs=4) as opool,
            tc.tile_pool(name="psum", bufs=6, space="PSUM") as ppool,
            tc.tile_pool(name="psum4", bufs=2, space="PSUM") as p4pool,
        ):
            # Two HWDGE rings (SP + ACT): alternate issue engine per DMA so
            # queue-push (DIRECT2D) overhead parallelizes across sequencers.
            dma_engines = [nc.sync, nc.scalar]
            n_dma = 0

            def dma(out, in_):
                nonlocal n_dma
                dma_engines[n_dma % 2].dma_start(out=out, in_=in_)
                n_dma += 1

            bt = bpool.tile([MB + 4, KS, MB], mm_dt, tag="band")
            dma(bt[:], bm[:])

            # Global edge input: padded rows [496, 516) of every image, one DMA.
            # SBUF layout [row 20, img 16, 516]; DRAM iterates row-outer to match.
            xe = epool.tile([KE, IMGS_PER_CORE, WP], mm_dt, tag="xe")
            dma(
                xe[:],
                bass.AP(
                    xh,
                    (NB * MB) * WP,
                    [[WP, KE], [HP * WP, IMGS_PER_CORE], [1, WP]],
                ),
            )
            # Global edge output accumulator [row 16, img 16, 512].
            oe = epool.tile([ME, IMGS_PER_CORE, W], F32, tag="oe")

            for img in range(IMGS_PER_CORE):
                xts = []
                for q in range(NB):
                    xt = xpool.tile([128, WP], mm_dt)
                    dma(xt[:, :], x[img, q * MB:q * MB + 128, :])
                    xts.append(xt)

                ot = opool.tile([MB, NB, W], F32, tag="o")
                for q in range(NB):
                    P = ppool.tile([MB, W], F32, tag="P")
                    for j in range(KS):
                        nc.tensor.matmul(
                            P[:MB, :],
                            bt[:128, j, :MB],
                            xts[q][:128, j:j + W],
                            start=(j == 0),
                            stop=(j == KS - 1),
                        )
                    nc.vector.tensor_copy(ot[:MB, q, :], P[:MB, :])

                # One ~1 MB store for rows [0, 496): DRAM iterates p-outer,
                # q-inner to match SBUF [p, q, w] -> DRAM row q*124 + p.
                dma(
                    bass.AP(
                        yh,
                        img * H * W,
                        [[W, MB], [MB * W, NB], [1, W]],
                    ),
                    ot[:],
                )

                # Edge block: output rows [496, 512) from padded rows [496, 516).
                P4 = p4pool.tile([ME, W], F32, tag="P4")
                for j in range(KS):
                    nc.tensor.matmul(
                        P4[:ME, :],
                        bt[:KE, j, :ME],
                        xe[:KE, img, j:j + W],
                        start=(j == 0),
                        stop=(j == KS - 1),
                    )
                nc.vector.tensor_copy(oe[:ME, img, :], P4[:ME, :])

            # One store for all images' edge rows [496, 512).
            dma(
                bass.AP(
                    yh,
                    (NB * MB) * W,
                    [[W, ME], [H * W, IMGS_PER_CORE], [1, W]],
                ),
                oe[:],
            )

    nc.compile()
    return nc


def kernel(X, kernel, stride, padding):
    assert int(stride) == 1 and int(padding) == 2
    X = np.asarray(X, dtype=np.float32)
    B, C, HH, WW = X.shape
    assert (B * C, HH, WW) == (N_CORES * IMGS_PER_CORE, H, W)

    if "nc" not in _CACHE:
        _CACHE["nc"] = build_nc()
    nc = _CACHE["nc"]

    band = build_bands(kernel)
    Xp = np.zeros((N_CORES, IMGS_PER_CORE, HP, WP), dtype=np.float32)
    Xp[:, :, 2:2 + H, 2:2 + W] = X.reshape(N_CORES, IMGS_PER_CORE, H, W)
    in_maps = [{"x": Xp[c], "bm": band} for c in range(N_CORES)]
    res = run_bass_kernel_spmd(
        nc, in_maps, core_ids=list(range(N_CORES)), **_CACHE.get("run_kwargs", {})
    )
    _CACHE["last_results"] = res
    out = np.stack([res.results[c]["y"] for c in range(N_CORES)], axis=0)
    return out.reshape(B, C, HH, WW).astype(np.float32)


# revision 12
# speedup vs baseline: 1.4609x; 1.4609x over previous
"""Depthwise 5x5 correlation (stride 1, pad 2) over X[4, 32, 512, 512] fp32,
with a single shared [5, 5] kernel, on 8 Trainium2 NeuronCores.

Strategy (pure data parallel): the 4*32 = 128 images are split 16 per core.
The input is zero-padded host-side to [516, 516] (pad 2 in H and W), so on
device the conv decomposes per kernel column j:
    O[h, w] = sum_j C_j[h, w],   C_j[h, w] = sum_k B_j[k, h] X'[h + k, w + j]
where B_j is a single banded-Toeplitz stationary matrix (B_j[k, m] =
kernel[k - m, j]); one TensorE matmul per (row-block, j), all five j's
accumulating into the same PSUM bank (start=True on j=0 zero-fills it), with
the W shift folded into the rhs read offset.

H is tiled into 4 uniform blocks of 124 output rows (each reading 128 padded
input rows) plus one 16-row edge block (reading 20 padded rows). The four
uniform blocks of an image share one SBUF output tile [124, 4, 512] written
back with a single ~1 MB DMA whose descriptors spread across all 16 SDMA
engines; the 16-row edges of all images are batched into one global in-DMA
and one global out-DMA. DMA issue alternates between the SP and ACT HWDGE
rings to parallelize queue-push overhead.

Matmuls run as float32r (single-pass relaxed fp32, 4x faster than strict fp32
on the PE, fp32 PSUM accumulate).
"""

import numpy as np

import concourse.bacc as bacc
import concourse.bass as bass
import concourse.mybir as mybir
import concourse.tile as tile
from concourse.bass_utils import run_bass_kernel_spmd

F32 = mybir.dt.float32
F32R = mybir.dt.float32r

N_CORES = 8
IMGS_PER_CORE = 16
H = W = 512
HP = H + 4
WP = W + 4
KS = 5

NB = 4           # uniform row blocks per image
MB = 124         # output rows per uniform block
ME = 16          # output rows in the edge block (rows 496..512)
KE = ME + KS - 1  # padded input rows the edge block reads

USE_F32R = True

_CACHE = {}


def build_bands(kern):
    """kern: [5, 5] fp32 -> [128, 5, 124] banded-Toeplitz stationary matrices,
    partition-major. B[k, j, m] = kern[k - m, j] for k - m in [0, 5).
    The edge block uses the [:20, :, :16] slice (same band structure)."""
    kern = np.asarray(kern, dtype=np.float32)
    B = np.zeros((MB + 4, KS, MB), dtype=np.float32)
    k_idx = np.arange(MB + 4)[:, None]
    m_idx = np.arange(MB)[None, :]
    tap = k_idx - m_idx
    valid = (tap >= 0) & (tap < KS)
    kk, mm = np.nonzero(valid)
    for j in range(KS):
        B[kk, j, mm] = kern[tap[kk, mm], j]
    return B


def build_nc():
    # float32r end-to-end on the matmul operand path (DRAM declaration, DMA,
    # SBUF tile, matmul input): walrus' BIR verifier requires the producer of
    # an FP32r matmul operand to emit FP32r. Same 4-byte fp32 bits on the wire.
    mm_dt = F32R if USE_F32R else F32
    nc = bacc.Bacc("TRN2", target_bir_lowering=False, debug=False)

    x = nc.dram_tensor("x", [IMGS_PER_CORE, HP, WP], mm_dt, kind="ExternalInput").ap()
    bm = nc.dram_tensor("bm", [MB + 4, KS, MB], mm_dt, kind="ExternalInput").ap()
    y = nc.dram_tensor("y", [IMGS_PER_CORE, H, W], F32, kind="ExternalOutput").ap()
    xh = x.tensor  # handle for raw-AP construction
    yh = y.tensor

    with tile.TileContext(nc) as tc:
        with (
            tc.tile_pool(name="bands", bufs=1) as bpool,
            tc.tile_pool(name="xin", bufs=12) as xpool,
            tc.tile_pool(name="edge", bufs=1) as epool,
            tc.tile_pool(name="out", bufs=4) as opool,
            tc.tile_pool(name="psum", bufs=6, space="PSUM") as ppool,
            tc.tile_pool(name="psum4", bufs=2, space="PSUM") as p4pool,
        ):
            # Two HWDGE rings (SP + ACT): alternate issue engine per DMA so
            # queue-push (DIRECT2D) overhead parallelizes across sequencers.
            dma_engines = [nc.sync, nc.scalar]
            n_dma = 0

            def dma(out, in_):
                nonlocal n_dma
                dma_engines[n_dma % 2].dma_start(out=out, in_=in_)
                n_dma += 1

            def dma_store(out, in_):
                # HWDGE stores land on SDMA engines 0-3 only (observed in
                # traces on both rings); SWDGE-issued stores spread across
                # all 16 engine slots and keep the big queue-push off the
                # SP/ACT sequencers.
                nc.gpsimd.dma_start(out=out, in_=in_)

            bt = bpool.tile([MB + 4, KS, MB], mm_dt, tag="band")
            dma(bt[:], bm[:])

            # Global edge input: padded rows [496, 516) of every image, one DMA.
            # SBUF layout [row 20, img 16, 516]; DRAM iterates row-outer to match.
            xe = epool.tile([KE, IMGS_PER_CORE, WP], mm_dt, tag="xe")
            dma(
                xe[:],
                bass.AP(
                    xh,
                    (NB * MB) * WP,
                    [[WP, KE], [HP * WP, IMGS_PER_CORE], [1, WP]],
                ),
            )
            # Global edge output accumulator [row 16, img 16, 512].
            oe = epool.tile([ME, IMGS_PER_CORE, W], F32, tag="oe")

            for img in range(IMGS_PER_CORE):
                xts = []
                for q in range(NB):
                    xt = xpool.tile([128, WP], mm_dt)
                    dma(xt[:, :], x[img, q * MB:q * MB + 128, :])
                    xts.append(xt)

                ot = opool.tile([MB, NB, W], F32, tag="o")
                for q in range(NB):
                    P = ppool.tile([MB, W], F32, tag="P")
                    for j in range(KS):
                        nc.tensor.matmul(
                            P[:MB, :],
                            bt[:128, j, :MB],
                            xts[q][:128, j:j + W],
                            start=(j == 0),
                            stop=(j == KS - 1),
                        )
                    nc.vector.tensor_copy(ot[:MB, q, :], P[:MB, :])

                # One ~1 MB store for rows [0, 496): DRAM iterates p-outer,
                # q-inner to match SBUF [p, q, w] -> DRAM row q*124 + p.
                dma_store(
                    bass.AP(
                        yh,
                        img * H * W,
                        [[W, MB], [MB * W, NB], [1, W]],
                    ),
                    ot[:],
                )

                # Edge block: output rows [496, 512) from padded rows [496, 516).
                P4 = p4pool.tile([ME, W], F32, tag="P4")
                for j in range(KS):
                    nc.tensor.matmul(
                        P4[:ME, :],
                        bt[:KE, j, :ME],
                        xe[:KE, img, j:j + W],
                        start=(j == 0),
                        stop=(j == KS - 1),
                    )
                nc.vector.tensor_copy(oe[:ME, img, :], P4[:ME, :])

            # One store for all images' edge rows [496, 512).
            dma_store(
                bass.AP(
                    yh,
                    (NB * MB) * W,
                    [[W, ME], [H * W, IMGS_PER_CORE], [1, W]],
                ),
                oe[:],
            )

    nc.compile()
    return nc


def kernel(X, kernel, stride, padding):
    assert int(stride) == 1 and int(padding) == 2
    X = np.asarray(X, dtype=np.float32)
    B, C, HH, WW = X.shape
    assert (B * C, HH, WW) == (N_CORES * IMGS_PER_CORE, H, W)

    if "nc" not in _CACHE:
        _CACHE["nc"] = build_nc()
    nc = _CACHE["nc"]

    band = build_bands(kernel)
    Xp = np.zeros((N_CORES, IMGS_PER_CORE, HP, WP), dtype=np.float32)
    Xp[:, :, 2:2 + H, 2:2 + W] = X.reshape(N_CORES, IMGS_PER_CORE, H, W)
    in_maps = [{"x": Xp[c], "bm": band} for c in range(N_CORES)]
    res = run_bass_kernel_spmd(
        nc, in_maps, core_ids=list(range(N_CORES)), **_CACHE.get("run_kwargs", {})
    )
    _CACHE["last_results"] = res
    out = np.stack([res.results[c]["y"] for c in range(N_CORES)], axis=0)
    return out.reshape(B, C, HH, WW).astype(np.float32)
